# revision 1
# baseline (speedup 1.0000x reference)
import sys
import types

import numpy as np
from contextlib import ExitStack

try:
    import antenv.axon_hooks  # noqa: F401
except ImportError:
    _m = types.ModuleType("antenv.axon_hooks")
    _m._HOOK = None

    def _set_hook(h, _m=_m):
        _m._HOOK = h

    def _get_hook(_m=_m):
        return _m._HOOK

    _m.set_axon_ntff_profile_hook = _set_hook
    _m.get_axon_ntff_profile_hook = _get_hook
    sys.modules["antenv.axon_hooks"] = _m
    try:
        import antenv

        antenv.axon_hooks = _m
    except ImportError:
        pass

import concourse.bass as bass
import concourse.bacc as bacc
import concourse.tile as tile
from concourse import mybir
from concourse.bass_utils import run_bass_kernel_spmd
from concourse.masks import make_identity

F32 = mybir.dt.float32
AF = mybir.ActivationFunctionType
OP = mybir.AluOpType

B, S, D, M = 32, 2048, 1024, 1024
NC = 8
BP = B // NC          # batches per core = 4
ST = S // 128         # s-tiles per batch = 16
LN_EPS = 1e-5

LAST_RESULT = None    # test.py reads exec_time_ns from here


def _build(eta_f: float, theta_f: float, bvs_pre: float):
    nc = bacc.Bacc("TRN2", target_bir_lowering=False)
    d = nc.declare_dram_parameter
    x_d = d("x", [BP * S, D], F32, False)
    mem_d = d("mem", [BP, M], F32, False)
    mom_d = d("mom", [BP, M], F32, False)
    wk_d = d("wk", [D, M], F32, False)
    wkT_d = d("wkT", [M, D], F32, False)
    w0_d = d("w0", [M, M], F32, False)
    w0T_d = d("w0T", [M, M], F32, False)
    w1_d = d("w1", [M, M], F32, False)
    w1T_d = d("w1T", [M, M], F32, False)
    wf_d = d("wf", [D + M, M], F32, False)
    wu_d = d("wu", [D + M, M], F32, False)
    rows_d = {}
    for n in ("bk", "b0", "b1", "g0", "g1", "lb0", "lb1", "bfv", "buv", "wvs"):
        rows_d[n] = d(n, [1, M], F32, False)
    outp_d = d("out_p", [BP, M], F32, True)
    outm_d = d("out_m", [BP, M], F32, True)

    with tile.TileContext(nc) as tc, ExitStack() as ctx:
        keep = ctx.enter_context(tc.tile_pool(name="keep", bufs=1))
        temps = ctx.enter_context(tc.tile_pool(name="temps", bufs=7))
        sc = ctx.enter_context(tc.tile_pool(name="sc", bufs=12))
        wch = ctx.enter_context(tc.tile_pool(name="wch", bufs=4))
        tp = ctx.enter_context(tc.tile_pool(name="tp", bufs=3))

        def kt(tag, shape=(BP, M)):
            return keep.tile(list(shape), F32, tag=tag, name=tag)

        def tmp():
            return temps.tile([BP, M], F32, tag="tmp", name="tmp")

        def sct(tag=None):
            return sc.tile([BP, 1], F32, tag="sc", name="sc")

        ident = kt("ident", (128, 128))
        make_identity(nc, ident[:])
        epsc = kt("epsc", (BP, 1))
        nc.gpsimd.memset(epsc[:], LN_EPS)

        cb = {}
        for n in rows_d:
            t = kt("cb_" + n)
            for p in range(BP):
                nc.sync.dma_start(t[p : p + 1, :], rows_d[n][0:1, :])
            cb[n] = t

        mem_sb = kt("mem")
        nc.sync.dma_start(mem_sb[:], mem_d[:])
        mom_sb = kt("mom")
        nc.sync.dma_start(mom_sb[:], mom_d[:])

        def transpose_4(src, ps_tp, tag, dst_pool=None):
            # [4, 1024] -> [128, 32]; chunk k lives at cols 4k:4k+4
            pool = dst_pool if dst_pool is not None else tp
            dst = pool.tile([128, 4 * (M // 128)], F32, tag=tag)
            for k in range(M // 128):
                pt = ps_tp.tile([128, BP], F32, tag="pt")
                nc.tensor.transpose(pt[:], src[:, 128 * k : 128 * (k + 1)],
                                    ident[0:BP, 0:BP])
                nc.scalar.copy(dst[:, 4 * k : 4 * k + 4], pt[:])
            return dst

        def mm_stream(lhsT_ap_fn, wdram, nk, ps_mm, evict):
            # out[b, n] = sum_k lhs[b, k] * W[k, n], W streamed in [128,1024] chunks
            pz0 = ps_mm.tile([BP, 512], F32, tag="pz0")
            pz1 = ps_mm.tile([BP, 512], F32, tag="pz1")
            for k in range(nk):
                ch = wch.tile([128, M], F32, tag="ch")
                nc.sync.dma_start(ch[:], wdram[128 * k : 128 * (k + 1), :])
                nc.tensor.matmul(pz0[:], lhsT_ap_fn(k), ch[:, 0:512],
                                 start=(k == 0), stop=(k == nk - 1))
                nc.tensor.matmul(pz1[:], lhsT_ap_fn(k), ch[:, 512:1024],
                                 start=(k == 0), stop=(k == nk - 1))
            evict(0, pz0)
            evict(1, pz1)

        def layer_forward(h_sb, w_dram, b_b, g_b, lb_b, ps_tp, ps_mm, li,
                          hT_tag=None, hT_pool=None, save=False):
            hT = transpose_4(h_sb, ps_tp, hT_tag or f"hT{li}", dst_pool=hT_pool)
            z_sb = tmp()

            def ev(half, pz):
                nc.vector.tensor_add(z_sb[:, 512 * half : 512 * half + 512], pz[:],
                                     b_b[:, 512 * half : 512 * half + 512])

            mm_stream(lambda k: hT[:, 4 * k : 4 * k + 4], w_dram, 8, ps_mm, ev)

            ssum = sct()
            nc.vector.tensor_reduce(ssum[:], z_sb[:], mybir.AxisListType.X, OP.add)
            nmean = sct()
            nc.scalar.mul(nmean[:], ssum[:], -1.0 / M)
            cen = tmp()
            nc.vector.tensor_scalar(cen[:], z_sb[:], nmean[:], None, OP.add)
            sq = tmp()
            vs = sct()
            nc.scalar.activation(sq[:], cen[:], AF.Square, accum_out=vs[:])
            std = sct()
            nc.scalar.activation(std[:], vs[:], AF.Sqrt, bias=epsc[:], scale=1.0 / M)
            rstd = kt(f"rstd{li}", (BP, 1)) if save else sct()
            nc.vector.reciprocal(rstd[:], std[:])
            xhat = kt(f"xhat{li}") if save else tmp()
            nc.vector.tensor_scalar(xhat[:], cen[:], rstd[:], None, OP.mult)
            yt = tmp()
            nc.vector.tensor_mul(yt[:], xhat[:], g_b[:])
            y_sb = kt(f"y{li}") if save else tmp()
            nc.vector.tensor_add(y_sb[:], yt[:], lb_b[:])
            sgy = tmp()
            nc.scalar.activation(sgy[:], y_sb[:], AF.Sigmoid)
            h_next = kt(f"h{li}") if save else tmp()
            nc.vector.tensor_mul(h_next[:], y_sb[:], sgy[:])
            return h_next, hT, xhat, y_sb, rstd

        # ---------- Phase A: forward MLP(mem) -> mo, then u, a, beta ----------
        with tc.tile_pool(name="pstp_a", bufs=2, space="PSUM") as ps_tp, \
             tc.tile_pool(name="psmm_a", bufs=2, space="PSUM") as ps_mm, \
             tc.tile_pool(name="rowp", bufs=1) as rowp:
            h1, memT, xhat0, y0, rstd0 = layer_forward(
                mem_sb, w0_d, cb["b0"], cb["g0"], cb["lb0"], ps_tp, ps_mm, 0,
                hT_tag="memT", hT_pool=keep, save=True)
            mo, _, xhat1, y1, rstd1 = layer_forward(
                h1, w1_d, cb["b1"], cb["g1"], cb["lb1"], ps_tp, ps_mm, 1, save=True)

            # kappa = mo . bk  (per batch)
            kap = kt("kap", (BP, 1))
            scr0 = tmp()
            nc.vector.tensor_mul(scr0[:], mo[:], cb["bk"][:])
            scr0b = tmp()
            nc.scalar.activation(scr0b[:], scr0[:], AF.Copy, accum_out=kap[:])
            # u = mo @ WkT
            moT = transpose_4(mo, ps_tp, "moT")
            u_sb = tmp()

            def ev_u(half, pz):
                nc.scalar.copy(u_sb[:, 512 * half : 512 * half + 512], pz[:])

            mm_stream(lambda k: moT[:, 4 * k : 4 * k + 4], wkT_d, 8, ps_mm, ev_u)

            # abrow[:, 0:D] = a = u/(B*S) - wvs_pre
            # abrow[:, D]   = beta = kappa/(B*S) - bvs_pre
            us = tmp()
            nc.scalar.mul(us[:], u_sb[:], 1.0 / (B * S))
            abrow = kt("abrow", (BP, D + 1))
            nc.vector.tensor_sub(abrow[:, 0:D], us[:], cb["wvs"][:])
            nc.scalar.activation(abrow[:, D : D + 1], kap[:], AF.Copy,
                                 bias=-bvs_pre, scale=1.0 / (B * S))

            # partition_broadcast input must start at partition 0 -> DMA-stage
            a_bc = []
            for b in range(BP):
                row = rowp.tile([1, D + 1], F32, tag=f"row{b}", name=f"row{b}")
                nc.sync.dma_start(row[:], abrow[b : b + 1, :])
                ab = kt(f"abc{b}", (128, D + 1))
                nc.gpsimd.partition_broadcast(ab[:], row[:])
                a_bc.append(ab)

        # ---------- Phase B: stream X ----------
        gx_sb = kt("gx")
        xsum_sb = kt("xsum")
        csum_sb = kt("csum", (BP, 1))
        with tc.tile_pool(name="pa", bufs=2, space="PSUM") as pa_p, \
             tc.tile_pool(name="pb", bufs=2, space="PSUM") as pb_p, \
             tc.tile_pool(name="pc", bufs=2, space="PSUM") as pc_p, \
             tc.tile_pool(name="xt", bufs=3) as xt_p, \
             tc.tile_pool(name="scr", bufs=2) as scr_p, \
             tc.tile_pool(name="scr2", bufs=1) as scr2_p, \
             tc.tile_pool(name="stg", bufs=1) as stg_p:
            for b in range(BP):
                pa = pa_p.tile([2, 512], F32, tag="pa")
                pb = pb_p.tile([2, 512], F32, tag="pb")
                pc = pc_p.tile([2, 2], F32, tag="pc")
                for t in range(ST):
                    r0 = b * S + t * 128
                    # xt cols: [0:D)=X  [D]=1.0  [D+1]=c
                    xt = xt_p.tile([128, D + 2], F32, tag="xt")
                    nc.sync.dma_start(xt[:, 0:D], x_d[r0 : r0 + 128, :])
                    nc.any.memset(xt[:, D : D + 1], 1.0)
                    scr = scr_p.tile([128, D + 1], F32, tag="scr")
                    nc.vector.tensor_mul(scr[:], xt[:, 0 : D + 1], a_bc[b][:])
                    scr2 = scr2_p.tile([128, D + 1], F32, tag="scr2")
                    nc.scalar.activation(scr2[:], scr[:], AF.Copy,
                                         accum_out=xt[:, D + 1 : D + 2])
                    nc.tensor.matmul(pa[:], xt[:, D : D + 2], xt[:, 0:512],
                                     start=(t == 0), stop=(t == ST - 1))
                    nc.tensor.matmul(pb[:], xt[:, D : D + 2], xt[:, 512:1024],
                                     start=(t == 0), stop=(t == ST - 1))
                    nc.tensor.matmul(pc[:], xt[:, D : D + 2], xt[:, D : D + 2],
                                     start=(t == 0), stop=(t == ST - 1))
                stage = stg_p.tile([2, D + 2], F32, tag="stage")
                nc.scalar.copy(stage[:, 0:512], pa[:])
                nc.scalar.copy(stage[:, 512:1024], pb[:])
                nc.scalar.copy(stage[:, 1024:1026], pc[:])
                # lhsT rows: p=0 -> ones, p=1 -> c
                nc.sync.dma_start(xsum_sb[b : b + 1, :], stage[0:1, 0:D])
                nc.sync.dma_start(gx_sb[b : b + 1, :], stage[1:2, 0:D])
                nc.sync.dma_start(csum_sb[b : b + 1, 0:1], stage[0:1, D + 1 : D + 2])

        # ---------- Phase C: dmo, backward, gates, update, output MLP ----------
        with tc.tile_pool(name="pstp_c", bufs=2, space="PSUM") as ps_tp, \
             tc.tile_pool(name="psmm_c", bufs=2, space="PSUM") as ps_mm:
            pooled = kt("pooled")
            nc.scalar.mul(pooled[:], xsum_sb[:], 1.0 / S)

            # dmo = gx @ Wk + csum * bk
            bkc = tmp()
            nc.vector.tensor_scalar(bkc[:], cb["bk"][:], csum_sb[:, 0:1], None, OP.mult)
            gxT = transpose_4(gx_sb, ps_tp, "gxT")
            dmo = kt("dmo")

            def ev_dmo(half, pz):
                nc.vector.tensor_add(dmo[:, 512 * half : 512 * half + 512], pz[:],
                                     bkc[:, 512 * half : 512 * half + 512])

            mm_stream(lambda k: gxT[:, 4 * k : 4 * k + 4], wk_d, 8, ps_mm, ev_dmo)

            # backward through the 2-layer MLP
            dcur = dmo
            for i in (1, 0):
                y_i = y1 if i == 1 else y0
                xh_i = xhat1 if i == 1 else xhat0
                rs_i = rstd1 if i == 1 else rstd0
                g_b = cb["g1"] if i == 1 else cb["g0"]
                wT_d = w1T_d if i == 1 else w0T_d

                sg = tmp()
                nc.scalar.activation(sg[:], y_i[:], AF.Sigmoid)
                t1 = tmp()
                nc.vector.tensor_mul(t1[:], y_i[:], sg[:])
                t2 = tmp()
                nc.vector.tensor_mul(t2[:], t1[:], sg[:])
                t3 = tmp()
                nc.vector.tensor_add(t3[:], sg[:], t1[:])
                t4 = tmp()
                nc.vector.tensor_sub(t4[:], t3[:], t2[:])       # silu'(y)
                dy = tmp()
                nc.vector.tensor_mul(dy[:], dcur[:], t4[:])
                dxh = tmp()
                nc.vector.tensor_mul(dxh[:], dy[:], g_b[:])

                rsum = sct()
                nc.vector.tensor_reduce(rsum[:], dxh[:], mybir.AxisListType.X, OP.add)
                nm1 = sct()
                nc.scalar.mul(nm1[:], rsum[:], -1.0 / M)
                junk = tmp()
                nc.vector.tensor_mul(junk[:], dxh[:], xh_i[:])
                rs2 = sct()
                junk2 = tmp()
                nc.scalar.activation(junk2[:], junk[:], AF.Copy, accum_out=rs2[:])
                nmh = sct()
                nc.scalar.mul(nmh[:], rs2[:], -1.0 / M)
                t5 = tmp()
                nc.vector.tensor_scalar(t5[:], xh_i[:], nmh[:], None, OP.mult)
                t6 = tmp()
                nc.vector.tensor_add(t6[:], dxh[:], t5[:])
                t7 = tmp()
                nc.vector.tensor_scalar(t7[:], t6[:], nm1[:], None, OP.add)
                dz = tmp()
                nc.vector.tensor_scalar(dz[:], t7[:], rs_i[:], None, OP.mult)

                dzT = transpose_4(dz, ps_tp, f"dzT{i}")
                dnext = kt(f"dh{i}")

                def ev_dh(half, pz, _dst=dnext):
                    nc.scalar.copy(_dst[:, 512 * half : 512 * half + 512], pz[:])

                mm_stream(lambda k: dzT[:, 4 * k : 4 * k + 4], wT_d, 8, ps_mm, ev_dh)
                dcur = dnext
            surprise = dcur

            # gates: gate_in = [pooled | mem]
            pooledT = transpose_4(pooled, ps_tp, "pooledT")

            def gate_lhsT(k):
                if k < 8:
                    return pooledT[:, 4 * k : 4 * k + 4]
                return memT[:, 4 * (k - 8) : 4 * (k - 8) + 4]

            def make_gate(w_dram, bias_b, tag):
                g_sb = kt(tag)

                def ev(half, pz):
                    tt = tmp()
                    nc.vector.tensor_add(tt[:, 0:512], pz[:],
                                         bias_b[:, 512 * half : 512 * half + 512])
                    nc.scalar.activation(g_sb[:, 512 * half : 512 * half + 512],
                                         tt[:, 0:512], AF.Sigmoid)

                mm_stream(gate_lhsT, w_dram, 16, ps_mm, ev)
                return g_sb

            forget_g = make_gate(wf_d, cb["bfv"], "fgate")
            update_g = make_gate(wu_d, cb["buv"], "ugate")

            # new_momentum = eta*mom + theta*surprise
            ta = tmp()
            nc.vector.tensor_scalar(ta[:], mom_sb[:], eta_f, None, OP.mult)
            tb = tmp()
            nc.vector.tensor_scalar(tb[:], surprise[:], theta_f, None, OP.mult)
            nm_sb = tmp()
            nc.vector.tensor_add(nm_sb[:], ta[:], tb[:])

            # new_memory = (1-forget)*mem + update*new_momentum
            tc1 = tmp()
            nc.vector.tensor_mul(tc1[:], forget_g[:], mem_sb[:])
            tc2 = tmp()
            nc.vector.tensor_sub(tc2[:], mem_sb[:], tc1[:])
            tc3 = tmp()
            nc.vector.tensor_mul(tc3[:], update_g[:], nm_sb[:])
            newmem = kt("newmem")
            nc.vector.tensor_add(newmem[:], tc2[:], tc3[:])

            # processed = MLP(new_memory)
            p1, _, _, _, _ = layer_forward(newmem, w0_d, cb["b0"], cb["g0"],
                                           cb["lb0"], ps_tp, ps_mm, 0,
                                           hT_tag="nmT")
            proc, _, _, _, _ = layer_forward(p1, w1_d, cb["b1"], cb["g1"],
                                             cb["lb1"], ps_tp, ps_mm, 1,
                                             hT_tag="p1T")

            nc.sync.dma_start(outp_d[:], proc[:])
            nc.sync.dma_start(outm_d[:], newmem[:])

    nc.finalize()
    return nc


def _prep(inputs):
    f = lambda k: np.ascontiguousarray(np.asarray(inputs[k], dtype=np.float32))
    X = f("inputs")
    mem = f("memory_state")
    mom = f("momentum_state")
    Wk, bk = f("Wk"), f("bk")
    Wv, bv = f("Wv"), f("bv")
    mem_W, mem_b = f("mem_W"), f("mem_b")
    ln_g, ln_b = f("ln_g"), f("ln_b")
    Wf, Wu = f("Wf"), f("Wu")
    bfv, buv = f("bf"), f("bu")
    eta_f = float(np.asarray(inputs["eta"]).reshape(-1)[0])
    theta_f = float(np.asarray(inputs["theta"]).reshape(-1)[0])

    bvs_pre = float(bv.sum()) / (B * S * M)
    wvs_pre = (Wv.sum(axis=1) / (B * S * M)).astype(np.float32).reshape(1, M)

    nc = _build(eta_f, theta_f, bvs_pre)

    shared = {
        "wk": Wk,
        "wkT": np.ascontiguousarray(Wk.T),
        "w0": np.ascontiguousarray(mem_W[0]),
        "w0T": np.ascontiguousarray(mem_W[0].T),
        "w1": np.ascontiguousarray(mem_W[1]),
        "w1T": np.ascontiguousarray(mem_W[1].T),
        "wf": Wf,
        "wu": Wu,
        "bk": bk.reshape(1, M),
        "b0": np.ascontiguousarray(mem_b[0]).reshape(1, M),
        "b1": np.ascontiguousarray(mem_b[1]).reshape(1, M),
        "g0": np.ascontiguousarray(ln_g[0]).reshape(1, M),
        "g1": np.ascontiguousarray(ln_g[1]).reshape(1, M),
        "lb0": np.ascontiguousarray(ln_b[0]).reshape(1, M),
        "lb1": np.ascontiguousarray(ln_b[1]).reshape(1, M),
        "bfv": bfv.reshape(1, M),
        "buv": buv.reshape(1, M),
        "wvs": wvs_pre,
    }
    in_maps = []
    for c in range(NC):
        m = dict(shared)
        m["x"] = np.ascontiguousarray(X[c * BP : (c + 1) * BP].reshape(BP * S, D))
        m["mem"] = np.ascontiguousarray(mem[c * BP : (c + 1) * BP])
        m["mom"] = np.ascontiguousarray(mom[c * BP : (c + 1) * BP])
        in_maps.append(m)
    return nc, in_maps


def kernel(**inputs):
    global LAST_RESULT
    nc, in_maps = _prep(inputs)
    res = run_bass_kernel_spmd(nc, in_maps, list(range(NC)))
    LAST_RESULT = res
    outs = res.results
    processed = np.concatenate([outs[c]["out_p"] for c in range(NC)], axis=0)
    new_memory = np.concatenate([outs[c]["out_m"] for c in range(NC)], axis=0)
    return processed.astype(np.float32), new_memory.astype(np.float32)

